# revision 1
# baseline (speedup 1.0000x reference)
"""Trainium2 Bass kernel for a GAT block.

Math (after algebraic simplification of the reference):
  h[b,f,n,k] = x[b,:,f,n] @ W[:,k] + bW[k]
  s2[b,f,n]  = h[b,f,n,:] @ a2 = v.x + c0   (s1/ab cancel inside softmax)
  d[b,f,n]   = softmax_n(s2)[n] * mask[n,n]
  out[b,k,f,n] = d[b,f,n] * h[b,f,n,k] = sum_c W[c,k] (x*d)[c,f,n] + bW[k] d[f,n]

Sharding: data-parallel over batch, 4 batches per core on 8 cores.

Device pipeline per (batch, 512-frame q-unit), shapes are [partitions, free]:
  1. xs [128, 3, 100]: partition = 4-frame quad (all 128 lanes busy)
  2. softmax on DVE/ACT -> dd128 [128, 100]; flatten DMA -> dd [32, 400]
  3. psum_dd [128, 400] = rep4.T @ dd   (PE replicates dd into 4 blocks)
  4. x4 [128, 400]: rows 32c+fsub = x[c], rows 96:128 = 1.0 (memset);
     x4s = x4 * psum_dd  (one DVE op: x*d rows 0:96, d rows 96:128)
  5. 16 matmuls into 2-bank psum tiles [128, 1024] (cols 0:400 and
     512:912): psum = wsel[tp].T @ x4s; wsel[tp] [128,128] selects fsubs
     {tp, 16+tp} and applies [W; bW] -> final out for 32 frames,
     rows = (2k + jj), cols (f', n)
  6. evict 2 tiles per op (DVE/ACT alternating) -> osb [128, 16, 400];
     2 half stores per q-unit ([128, 3200], 12.8KB descriptors)
"""

import sys

if "/opt/trn_rl_repo" not in sys.path:
    sys.path.insert(0, "/opt/trn_rl_repo")

import numpy as np

B, C, F, N, H = 32, 3, 2048, 25, 64
NCORES = 8
BPC = B // NCORES   # batches per core
QF = 512            # frames per q-unit
NQ = F // QF        # q-units per batch
FSUB = 16           # frames per fsub row
NS = QF // FSUB     # 32 fsub rows per q-unit
FN = F * N
TW = FSUB * N       # 400, columns per tile
NT = NS // 2        # 16 tiles (of 32 frames) per q-unit
QW = 4 * N          # 100, columns per frame-quad row

# matmul operand dtype: "f32" (exact) or "f32r" (~2e-4 rel err, 4x faster PE)
MM_DTYPE = "f32"

_NC_CACHE = {}


def _build_nc():
    import concourse.bass as bass
    import concourse.bacc as bacc
    import concourse.tile as tile
    from concourse import mybir

    f32 = mybir.dt.float32
    mmdt = f32 if MM_DTYPE == "f32" else mybir.dt.float32r
    MULT = mybir.AluOpType.mult
    ADD = mybir.AluOpType.add
    AX = mybir.AxisListType.X
    EXP = mybir.ActivationFunctionType.Exp

    nc = bacc.Bacc()
    x_d = nc.declare_dram_parameter("x", [BPC, C, F, N], f32, isOutput=False)
    wsel_d = nc.declare_dram_parameter("wsel", [128, NT, 128], mmdt, isOutput=False)
    rep4_d = nc.declare_dram_parameter("rep4", [NS, 128], f32, isOutput=False)
    v_d = nc.declare_dram_parameter("v_pp", [128, C], f32, isOutput=False)
    c0_d = nc.declare_dram_parameter("c0_pp", [128, 1], f32, isOutput=False)
    md_d = nc.declare_dram_parameter("mdq", [128, QW], f32, isOutput=False)
    out_d = nc.declare_dram_parameter("out", [BPC, H, F, N], f32, isOutput=True)

    with tile.TileContext(nc) as tc:
        with (
            tc.tile_pool(name="singles", bufs=1) as singles,
            tc.tile_pool(name="xs", bufs=3) as xs_pool,
            tc.tile_pool(name="sm", bufs=3) as sm_pool,
            tc.tile_pool(name="x4", bufs=3) as x4_pool,
            tc.tile_pool(name="osb", bufs=3) as osb_pool,
            tc.tile_pool(name="ps", bufs=7, space="PSUM") as ps_pool,
            tc.tile_pool(name="psd", bufs=1, space="PSUM") as psd_pool,
        ):
            wsel_sb = singles.tile([128, NT, 128], mmdt)
            nc.sync.dma_start(out=wsel_sb[:], in_=wsel_d[:, :, :])
            rep4_sb = singles.tile([NS, 128], f32)
            nc.sync.dma_start(out=rep4_sb[:], in_=rep4_d[:, :])
            v_sb = singles.tile([128, C], f32)
            nc.sync.dma_start(out=v_sb[:], in_=v_d[:, :])
            c0_sb = singles.tile([128, 1], f32)
            nc.sync.dma_start(out=c0_sb[:], in_=c0_d[:, :])
            md_sb = singles.tile([128, QW], f32)
            nc.sync.dma_start(out=md_sb[:], in_=md_d[:, :])

            units = [(b, q) for b in range(BPC) for q in range(NQ)]

            def emit_loads(u):
                """Emit the two input DMAs for unit u; return (xs, x4)."""
                b, q = u
                f0 = q * QF
                base = x_d[b, :, f0 : f0 + 1, :]  # for offset only
                xs = xs_pool.tile([128, C, QW], f32)
                src = bass.AP(
                    tensor=base.tensor,
                    offset=base.offset,
                    ap=[[QW, 128], [FN, C], [1, QW]],
                )
                nc.scalar.dma_start(out=xs[:], in_=src)
                x4 = x4_pool.tile([128, TW], f32, tag="x4")
                nc.vector.memset(x4[96:128, :], 1.0)
                src4 = bass.AP(
                    tensor=base.tensor,
                    offset=base.offset,
                    ap=[[FN, C], [TW, NS], [1, TW]],
                )
                nc.sync.dma_start(out=x4[0:96, :], in_=src4)
                return xs, x4

            pending = emit_loads(units[0])
            for ui, u in enumerate(units):
                b, q = u
                f0 = q * QF
                xs, x4 = pending
                if ui + 1 < len(units):
                    pending = emit_loads(units[ui + 1])
                # ---- 2. softmax in frame-quad layout -> dd128 [128, 100]
                t = sm_pool.tile([128, QW], f32, tag="t")
                nc.vector.tensor_scalar(
                    out=t[:],
                    in0=xs[:, 2, :],
                    scalar1=v_sb[:, 2:3],
                    scalar2=c0_sb[:, :],
                    op0=MULT,
                    op1=ADD,
                )
                for c in (1, 0):
                    nc.vector.scalar_tensor_tensor(
                        out=t[:],
                        in0=xs[:, c, :],
                        scalar=v_sb[:, c : c + 1],
                        in1=t[:],
                        op0=MULT,
                        op1=ADD,
                    )
                e = sm_pool.tile([128, QW], f32, tag="e")
                nc.scalar.activation(out=e[:], in_=t[:], func=EXP)
                ev = e[:].rearrange("p (a b) -> p a b", b=N)
                z = sm_pool.tile([128, 4], f32, tag="z")
                nc.vector.reduce_sum(out=z[:], in_=ev, axis=AX)
                r = sm_pool.tile([128, 4], f32, tag="r")
                nc.vector.reciprocal(out=r[:], in_=z[:])
                em = sm_pool.tile([128, QW], f32, tag="em")
                nc.vector.tensor_tensor(out=em[:], in0=e[:], in1=md_sb[:], op=MULT)
                dd128 = sm_pool.tile([128, QW], f32, tag="dd128")
                rr = r[:, :]
                r_bc = bass.AP(
                    tensor=rr.tensor,
                    offset=rr.offset,
                    ap=[rr.ap[0], [1, 4], [0, N]],
                )
                nc.vector.tensor_tensor(out=dd128[:], in0=em[:], in1=r_bc, op=MULT)
                # flatten [128, 100] -> [32, 400]
                dd = sm_pool.tile([NS, TW], f32, tag="dd")
                ddv = dd[:, :]
                dst = bass.AP(
                    tensor=ddv.tensor,
                    offset=ddv.offset,
                    ap=[ddv.ap[0], [QW, 4], [1, QW]],
                )
                nc.scalar.dma_start(out=dst, in_=dd128[:])
                # ---- 3. psum_dd [128, 400] = rep4.T @ dd
                pdd = psd_pool.tile([128, TW], f32, tag="pdd")
                nc.tensor.matmul(
                    pdd[:, :], rep4_sb[:], dd[:], start=True, stop=True
                )
                # ---- 4. x4s = x4 * psum_dd
                x4s = x4_pool.tile([128, TW], mmdt, tag="x4s")
                nc.vector.tensor_tensor(
                    out=x4s[:], in0=x4[:], in1=pdd[:], op=MULT
                )
                # ---- 5./6. 16 matmuls + evictions + stores
                osb = osb_pool.tile([128, NT, TW], f32)
                for tp in range(NT):
                    ph = ps_pool.tile([128, TW], f32, tag="ph")
                    nc.tensor.matmul(
                        ph[:, :],
                        wsel_sb[:, tp, :],
                        x4s[:, :],
                        start=True,
                        stop=True,
                    )
                    if tp % 3 == 0:
                        nc.vector.tensor_copy(osb[:, tp, :], ph[:, :])
                    else:
                        nc.scalar.copy(osb[:, tp, :], ph[:, :])
                    if tp % 8 == 7:
                        hh = tp // 8
                        osl = out_d[b, :, f0 : f0 + 1, :]
                        dst = bass.AP(
                            tensor=osl.tensor,
                            offset=osl.offset + hh * 8 * TW,
                            ap=[[FN, H], [16 * TW, 2], [1, 8 * TW]],
                        )
                        eng = nc.sync if hh == 0 else nc.scalar
                        eng.dma_start(
                            out=dst,
                            in_=osb[:, 8 * hh : 8 * (hh + 1), :],
                        )
    nc.compile()
    return nc


def _get_nc():
    if "nc" not in _NC_CACHE:
        _NC_CACHE["nc"] = _build_nc()
    return _NC_CACHE["nc"]


def _make_in_maps(x, mask, W, bW, a1, a2, ab):
    x = np.ascontiguousarray(np.asarray(x, np.float32))
    mask = np.asarray(mask, np.float32)
    W = np.asarray(W, np.float32)
    bW = np.asarray(bW, np.float32)
    a2 = np.asarray(a2, np.float32)

    v = (W @ a2).astype(np.float32)                    # [C]
    c0 = np.float32(bW @ a2)
    md = np.diag(mask).astype(np.float32)              # [N]

    # wsel[row = 32 c + fsub, tp, col = 2 k + jj]:
    #   delta[fsub == tp + 16 jj] * (W[c, k] if c < 3 else bW[k])
    # (column order (k, jj)-interleaved so the store DMA is affine)
    wsel = np.zeros((128, NT, 128), np.float32)
    cols = np.arange(H)
    for tp in range(NT):
        for jj in range(2):
            fsub = tp + 16 * jj
            for c in range(3):
                wsel[32 * c + fsub, tp, 2 * cols + jj] = W[c]
            wsel[96 + fsub, tp, 2 * cols + jj] = bW
    rep4 = np.zeros((NS, 128), np.float32)
    for blk in range(4):
        rep4[:, 32 * blk : 32 * (blk + 1)] = np.eye(NS, dtype=np.float32)
    v_pp = np.tile(v[None, :], (128, 1)).astype(np.float32)
    c0_pp = np.full((128, 1), c0, np.float32)
    mdq = np.tile(md[None, :], (128, 4)).astype(np.float32)

    in_maps = []
    for cix in range(NCORES):
        in_maps.append(
            {
                "x": np.ascontiguousarray(x[cix * BPC : (cix + 1) * BPC]),
                "wsel": wsel,
                "rep4": rep4,
                "v_pp": v_pp,
                "c0_pp": c0_pp,
                "mdq": mdq,
            }
        )
    return in_maps


def run(x, mask, W, bW, a1, a2, ab, **run_kwargs):
    from concourse.bass_utils import run_bass_kernel_spmd

    nc = _get_nc()
    in_maps = _make_in_maps(x, mask, W, bW, a1, a2, ab)
    res = run_bass_kernel_spmd(nc, in_maps, core_ids=list(range(NCORES)), **run_kwargs)
    out = np.concatenate([res.results[i]["out"] for i in range(NCORES)], axis=0)
    return out, res


def kernel(x, mask, W, bW, a1, a2, ab):
    out, _ = run(x, mask, W, bW, a1, a2, ab)
    return out



# revision 7
# speedup vs baseline: 1.1979x; 1.1979x over previous
"""Trainium2 Bass kernel for a GAT block.

Math (after algebraic simplification of the reference):
  h[b,f,n,k] = x[b,:,f,n] @ W[:,k] + bW[k]
  s2[b,f,n]  = h[b,f,n,:] @ a2 = v.x + c0   (s1/ab cancel inside softmax)
  d[b,f,n]   = softmax_n(s2)[n] * mask[n,n]
  out[b,k,f,n] = d[b,f,n] * h[b,f,n,k] = sum_c W[c,k] (x*d)[c,f,n] + bW[k] d[f,n]

Sharding: data-parallel over batch, 4 batches per core on 8 cores.

Device pipeline per (batch, 512-frame q-unit), shapes are [partitions, free]:
  1. xs [128, 3, 100]: partition = 4-frame quad (all 128 lanes busy)
  2. softmax on DVE/ACT -> dd128 [128, 100]; flatten DMA -> dd [32, 400]
  3. psum_dd [128, 400] = rep4.T @ dd   (PE replicates dd into 4 blocks)
  4. x4 [128, 400]: rows 32c+fsub = x[c], rows 96:128 = 1.0 (memset);
     x4s = x4 * psum_dd  (one DVE op: x*d rows 0:96, d rows 96:128)
  5. 16 matmuls into 2-bank psum tiles [128, 1024] (cols 0:400 and
     512:912): psum = wsel[tp].T @ x4s; wsel[tp] [128,128] selects fsubs
     {tp, 16+tp} and applies [W; bW] -> final out for 32 frames,
     rows = (2k + jj), cols (f', n)
  6. evict 2 tiles per op (DVE/ACT alternating) -> osb [128, 16, 400];
     2 half stores per q-unit ([128, 3200], 12.8KB descriptors)
"""

import sys

if "/opt/trn_rl_repo" not in sys.path:
    sys.path.insert(0, "/opt/trn_rl_repo")

import numpy as np

B, C, F, N, H = 32, 3, 2048, 25, 64
NCORES = 8
BPC = B // NCORES   # batches per core
QF = 512            # frames per q-unit
NQ = F // QF        # q-units per batch
FSUB = 16           # frames per fsub row
NS = QF // FSUB     # 32 fsub rows per q-unit
FN = F * N
TW = FSUB * N       # 400, columns per tile
NT = NS // 2        # 16 tiles (of 32 frames) per q-unit
QW = 4 * N          # 100, columns per frame-quad row

# matmul operand dtype: "f32" (exact) or "f32r" (~2e-4 rel err, 4x faster PE)
MM_DTYPE = "f32r"

_NC_CACHE = {}


def _build_nc():
    import concourse.bass as bass
    import concourse.bacc as bacc
    import concourse.tile as tile
    from concourse import mybir

    f32 = mybir.dt.float32
    bf16 = mybir.dt.bfloat16
    mmdt = f32 if MM_DTYPE == "f32" else mybir.dt.float32r
    MULT = mybir.AluOpType.mult
    ADD = mybir.AluOpType.add
    AX = mybir.AxisListType.X
    EXP = mybir.ActivationFunctionType.Exp

    nc = bacc.Bacc()
    x_d = nc.declare_dram_parameter("x", [BPC, C, F, N], f32, isOutput=False)
    wsel_d = nc.declare_dram_parameter("wsel", [128, NT, 128], mmdt, isOutput=False)
    rep4_d = nc.declare_dram_parameter("rep4", [NS, 128], f32, isOutput=False)
    v_d = nc.declare_dram_parameter("v_pp", [128, C], f32, isOutput=False)
    c0_d = nc.declare_dram_parameter("c0_pp", [128, 1], f32, isOutput=False)
    md_d = nc.declare_dram_parameter("mdq", [128, QW], f32, isOutput=False)
    out_d = nc.declare_dram_parameter("out", [BPC, H, F, N], bf16, isOutput=True)

    with tile.TileContext(nc) as tc:
        with (
            tc.tile_pool(name="singles", bufs=1) as singles,
            tc.tile_pool(name="xs", bufs=3) as xs_pool,
            tc.tile_pool(name="sm", bufs=3) as sm_pool,
            tc.tile_pool(name="x4", bufs=3) as x4_pool,
            tc.tile_pool(name="osb", bufs=3) as osb_pool,
            tc.tile_pool(name="ps", bufs=7, space="PSUM") as ps_pool,
            tc.tile_pool(name="psd", bufs=1, space="PSUM") as psd_pool,
        ):
            wsel_sb = singles.tile([128, NT, 128], mmdt)
            nc.sync.dma_start(out=wsel_sb[:], in_=wsel_d[:, :, :])
            rep4_sb = singles.tile([NS, 128], f32)
            nc.sync.dma_start(out=rep4_sb[:], in_=rep4_d[:, :])
            v_sb = singles.tile([128, C], f32)
            nc.sync.dma_start(out=v_sb[:], in_=v_d[:, :])
            c0_sb = singles.tile([128, 1], f32)
            nc.sync.dma_start(out=c0_sb[:], in_=c0_d[:, :])
            md_sb = singles.tile([128, QW], f32)
            nc.sync.dma_start(out=md_sb[:], in_=md_d[:, :])

            units = [(b, q) for b in range(BPC) for q in range(NQ)]

            def emit_loads(u):
                """Emit the two input DMAs for unit u; return (xs, x4)."""
                b, q = u
                f0 = q * QF
                base = x_d[b, :, f0 : f0 + 1, :]  # for offset only
                xs = xs_pool.tile([128, C, QW], f32)
                src = bass.AP(
                    tensor=base.tensor,
                    offset=base.offset,
                    ap=[[QW, 128], [FN, C], [1, QW]],
                )
                nc.scalar.dma_start(out=xs[:], in_=src)
                x4 = x4_pool.tile([128, TW], f32, tag="x4")
                nc.vector.memset(x4[96:128, :], 1.0)
                src4 = bass.AP(
                    tensor=base.tensor,
                    offset=base.offset,
                    ap=[[FN, C], [TW, NS], [1, TW]],
                )
                nc.sync.dma_start(out=x4[0:96, :], in_=src4)
                return xs, x4

            pending = emit_loads(units[0])
            for ui, u in enumerate(units):
                b, q = u
                f0 = q * QF
                xs, x4 = pending
                if ui + 1 < len(units):
                    pending = emit_loads(units[ui + 1])
                # ---- 2. softmax in frame-quad layout -> dd128 [128, 100]
                t = sm_pool.tile([128, QW], f32, tag="t")
                nc.vector.tensor_scalar(
                    out=t[:],
                    in0=xs[:, 2, :],
                    scalar1=v_sb[:, 2:3],
                    scalar2=c0_sb[:, :],
                    op0=MULT,
                    op1=ADD,
                )
                for c in (1, 0):
                    nc.vector.scalar_tensor_tensor(
                        out=t[:],
                        in0=xs[:, c, :],
                        scalar=v_sb[:, c : c + 1],
                        in1=t[:],
                        op0=MULT,
                        op1=ADD,
                    )
                e = sm_pool.tile([128, QW], f32, tag="e")
                nc.scalar.activation(out=e[:], in_=t[:], func=EXP)
                ev = e[:].rearrange("p (a b) -> p a b", b=N)
                z = sm_pool.tile([128, 4], f32, tag="z")
                nc.vector.reduce_sum(out=z[:], in_=ev, axis=AX)
                r = sm_pool.tile([128, 4], f32, tag="r")
                nc.vector.reciprocal(out=r[:], in_=z[:])
                em = sm_pool.tile([128, QW], f32, tag="em")
                nc.vector.tensor_tensor(out=em[:], in0=e[:], in1=md_sb[:], op=MULT)
                dd128 = sm_pool.tile([128, QW], f32, tag="dd128")
                rr = r[:, :]
                r_bc = bass.AP(
                    tensor=rr.tensor,
                    offset=rr.offset,
                    ap=[rr.ap[0], [1, 4], [0, N]],
                )
                nc.vector.tensor_tensor(out=dd128[:], in0=em[:], in1=r_bc, op=MULT)
                # flatten [128, 100] -> [32, 400]
                dd = sm_pool.tile([NS, TW], f32, tag="dd")
                ddv = dd[:, :]
                dst = bass.AP(
                    tensor=ddv.tensor,
                    offset=ddv.offset,
                    ap=[ddv.ap[0], [QW, 4], [1, QW]],
                )
                nc.scalar.dma_start(out=dst, in_=dd128[:])
                # ---- 3. psum_dd [128, 400] = rep4.T @ dd
                pdd = psd_pool.tile([128, TW], f32, tag="pdd")
                nc.tensor.matmul(
                    pdd[:, :], rep4_sb[:], dd[:], start=True, stop=True
                )
                # ---- 4. x4s = x4 * psum_dd
                x4s = x4_pool.tile([128, TW], mmdt, tag="x4s")
                nc.vector.tensor_tensor(
                    out=x4s[:], in0=x4[:], in1=pdd[:], op=MULT
                )
                # ---- 5./6. 16 matmuls + evictions + stores
                osb = osb_pool.tile([128, NT, TW], bf16)
                for tp in range(NT):
                    ph = ps_pool.tile([128, TW], f32, tag="ph")
                    nc.tensor.matmul(
                        ph[:, :],
                        wsel_sb[:, tp, :],
                        x4s[:, :],
                        start=True,
                        stop=True,
                    )
                    if tp % 3 == 0:
                        nc.scalar.copy(osb[:, tp, :], ph[:, :])
                    else:
                        nc.vector.tensor_copy(osb[:, tp, :], ph[:, :])
                    if tp % 8 == 7:
                        hh = tp // 8
                        osl = out_d[b, :, f0 : f0 + 1, :]
                        dst = bass.AP(
                            tensor=osl.tensor,
                            offset=osl.offset + hh * 8 * TW,
                            ap=[[FN, H], [16 * TW, 2], [1, 8 * TW]],
                        )
                        eng = nc.sync if hh == 0 else nc.scalar
                        eng.dma_start(
                            out=dst,
                            in_=osb[:, 8 * hh : 8 * (hh + 1), :],
                        )
    nc.compile()
    return nc


def _get_nc():
    if "nc" not in _NC_CACHE:
        _NC_CACHE["nc"] = _build_nc()
    return _NC_CACHE["nc"]


def _make_in_maps(x, mask, W, bW, a1, a2, ab):
    x = np.ascontiguousarray(np.asarray(x, np.float32))
    mask = np.asarray(mask, np.float32)
    W = np.asarray(W, np.float32)
    bW = np.asarray(bW, np.float32)
    a2 = np.asarray(a2, np.float32)

    v = (W @ a2).astype(np.float32)                    # [C]
    c0 = np.float32(bW @ a2)
    md = np.diag(mask).astype(np.float32)              # [N]

    # wsel[row = 32 c + fsub, tp, col = 2 k + jj]:
    #   delta[fsub == tp + 16 jj] * (W[c, k] if c < 3 else bW[k])
    # (column order (k, jj)-interleaved so the store DMA is affine)
    wsel = np.zeros((128, NT, 128), np.float32)
    cols = np.arange(H)
    for tp in range(NT):
        for jj in range(2):
            fsub = tp + 16 * jj
            for c in range(3):
                wsel[32 * c + fsub, tp, 2 * cols + jj] = W[c]
            wsel[96 + fsub, tp, 2 * cols + jj] = bW
    rep4 = np.zeros((NS, 128), np.float32)
    for blk in range(4):
        rep4[:, 32 * blk : 32 * (blk + 1)] = np.eye(NS, dtype=np.float32)
    v_pp = np.tile(v[None, :], (128, 1)).astype(np.float32)
    c0_pp = np.full((128, 1), c0, np.float32)
    mdq = np.tile(md[None, :], (128, 4)).astype(np.float32)

    in_maps = []
    for cix in range(NCORES):
        in_maps.append(
            {
                "x": np.ascontiguousarray(x[cix * BPC : (cix + 1) * BPC]),
                "wsel": wsel,
                "rep4": rep4,
                "v_pp": v_pp,
                "c0_pp": c0_pp,
                "mdq": mdq,
            }
        )
    return in_maps


def run(x, mask, W, bW, a1, a2, ab, **run_kwargs):
    from concourse.bass_utils import run_bass_kernel_spmd

    nc = _get_nc()
    in_maps = _make_in_maps(x, mask, W, bW, a1, a2, ab)
    res = run_bass_kernel_spmd(nc, in_maps, core_ids=list(range(NCORES)), **run_kwargs)
    out = np.concatenate(
        [np.asarray(res.results[i]["out"]).astype(np.float32) for i in range(NCORES)],
        axis=0,
    )
    return out, res


def kernel(x, mask, W, bW, a1, a2, ab):
    out, _ = run(x, mask, W, bW, a1, a2, ab)
    return out



# revision 8
# speedup vs baseline: 1.4797x; 1.2353x over previous
"""Trainium2 Bass kernel for a GAT block.

Math (after algebraic simplification of the reference):
  h[b,f,n,k] = x[b,:,f,n] @ W[:,k] + bW[k]
  s2[b,f,n]  = h[b,f,n,:] @ a2 = v.x + const   (s1/ab/const cancel in softmax)
  d[b,f,n]   = softmax_n(s2)[n] * mask[n,n]
  out[b,k,f,n] = d[b,f,n] * h[b,f,n,k] = sum_c W[c,k] (x*d)[c,f,n] + bW[k] d[f,n]

Sharding: data-parallel over batch, 4 batches per core on 8 cores.

Device pipeline per (batch, 512-frame q-unit), shapes are [partitions, free]:
  1. x4 [128, 400] bf16: rows 32c+s = x[c] for fsub s (16 frames x 25),
     rows 96:128 = 1.0 (gpsimd memset).
  2. s2p [32, 400] = vsel.T @ x4 on PE (vsel[32c+s, s] = v[c], rows 96+: 0).
  3. softmax: e = exp(s2p) (ACT, PSUM src) -> z = rowsum25 (DVE) ->
     r = 1/z (DVE) -> em = e*md400 (DVE 2x bf16) -> dd32 = em*r_bc (DVE).
  4. pdd [128, 400] = rep4.T @ dd32 (PE replicates d into 4 row blocks);
     x4s = x4 * pdd (DVE, bf16 out): x*d rows 0:96, d rows 96:128.
  5. 16 matmuls, constant-per-tp stationary wsel[tp] [128,128] bf16 (FWL):
     psum rows (2k+jj), cols (fi, n); pairs share a 2-bank psum tile
     [128, 1024] (cols 0:400 and 512:912).
  6. evictions (DVE/ACT mix): strided [128, 2x400] psum -> osb bf16;
     one store per q-unit: osb [128, 6400] -> out[b, :, f0:f0+512, :].

All PE operands are bf16 (fast weight loads); output is bf16, upcast to
fp32 on host (rel err ~4e-3 « 2e-2 tolerance).
"""

import sys

if "/opt/trn_rl_repo" not in sys.path:
    sys.path.insert(0, "/opt/trn_rl_repo")

import numpy as np
import ml_dtypes

B, C, F, N, H = 32, 3, 2048, 25, 64
NCORES = 8
BPC = B // NCORES   # batches per core
QF = 512            # frames per q-unit
NQ = F // QF        # q-units per batch
FSUB = 16           # frames per fsub row
NS = QF // FSUB     # 32 fsub rows per q-unit
FN = F * N
TW = FSUB * N       # 400, columns per tile
NT = NS // 2        # 16 matmuls (of 32 frames) per q-unit
NG = NT // 2        # 8 psum tile-pairs per q-unit

# evictions (of the 8 tile-pairs per unit) routed to DVE; rest go to ACT
DVE_TGS = (1, 4, 6)

_NC_CACHE = {}


def _build_nc():
    import concourse.bass as bass
    import concourse.bacc as bacc
    import concourse.tile as tile
    from concourse import mybir

    f32 = mybir.dt.float32
    bf16 = mybir.dt.bfloat16
    MULT = mybir.AluOpType.mult
    AX = mybir.AxisListType.X
    EXP = mybir.ActivationFunctionType.Exp

    nc = bacc.Bacc()
    x_d = nc.declare_dram_parameter("x", [BPC, C, F, N], bf16, isOutput=False)
    wsel_d = nc.declare_dram_parameter("wsel", [128, NT, 128], bf16, isOutput=False)
    rep4_d = nc.declare_dram_parameter("rep4", [NS, 128], bf16, isOutput=False)
    vsel_d = nc.declare_dram_parameter("vsel", [128, NS], bf16, isOutput=False)
    md_d = nc.declare_dram_parameter("md400", [NS, TW], bf16, isOutput=False)
    out_d = nc.declare_dram_parameter("out", [BPC, H, F, N], bf16, isOutput=True)

    with tile.TileContext(nc) as tc:
        with (
            tc.tile_pool(name="singles", bufs=1) as singles,
            tc.tile_pool(name="x4", bufs=3) as x4_pool,
            tc.tile_pool(name="sm", bufs=3) as sm_pool,
            tc.tile_pool(name="x4s", bufs=2) as x4s_pool,
            tc.tile_pool(name="osb", bufs=3) as osb_pool,
            tc.tile_pool(name="ps", bufs=2, space="PSUM") as ps_pool,
            tc.tile_pool(name="psd", bufs=1, space="PSUM") as psd_pool,
            tc.tile_pool(name="pss", bufs=2, space="PSUM") as pss_pool,
        ):
            wsel_sb = singles.tile([128, NT, 128], bf16)
            nc.sync.dma_start(out=wsel_sb[:], in_=wsel_d[:, :, :])
            rep4_sb = singles.tile([NS, 128], bf16)
            nc.sync.dma_start(out=rep4_sb[:], in_=rep4_d[:, :])
            vsel_sb = singles.tile([128, NS], bf16)
            nc.sync.dma_start(out=vsel_sb[:], in_=vsel_d[:, :])
            md_sb = singles.tile([NS, TW], bf16)
            nc.sync.dma_start(out=md_sb[:], in_=md_d[:, :])

            units = [(b, q) for b in range(BPC) for q in range(NQ)]
            NU = len(units)

            def emit_load(u):
                """x4 [128, 400] bf16: rows 0:96 from HBM, rows 96:128 = 1."""
                b, q = u
                f0 = q * QF
                base = x_d[b, :, f0 : f0 + 1, :]  # for offset only
                x4 = x4_pool.tile([128, TW], bf16, tag="x4")
                nc.gpsimd.memset(x4[96:128, :], 1.0)
                src4 = bass.AP(
                    tensor=base.tensor,
                    offset=base.offset,
                    ap=[[FN, C], [TW, NS], [1, TW]],
                )
                nc.sync.dma_start(out=x4[0:96, :], in_=src4)
                return x4

            def emit_s2(x4):
                """s2 matmul + softmax chain -> dd32 [32, 400] bf16."""
                s2p = pss_pool.tile([NS, TW], f32, tag="s2p")
                nc.tensor.matmul(
                    s2p[:, :], vsel_sb[:], x4[:], start=True, stop=True
                )
                e = sm_pool.tile([NS, TW], bf16, tag="e")
                nc.scalar.activation(out=e[:], in_=s2p[:], func=EXP)
                ev = e[:].rearrange("p (a b) -> p a b", b=N)
                z = sm_pool.tile([NS, FSUB], f32, tag="z")
                nc.vector.reduce_sum(out=z[:], in_=ev, axis=AX)
                r = sm_pool.tile([NS, FSUB], f32, tag="r")
                nc.vector.reciprocal(out=r[:], in_=z[:])
                em = sm_pool.tile([NS, TW], bf16, tag="em")
                nc.vector.tensor_tensor(out=em[:], in0=e[:], in1=md_sb[:], op=MULT)
                dd32 = sm_pool.tile([NS, TW], bf16, tag="dd32")
                rr = r[:, :]
                r_bc = bass.AP(
                    tensor=rr.tensor,
                    offset=rr.offset,
                    ap=[rr.ap[0], [1, FSUB], [0, N]],
                )
                nc.vector.tensor_tensor(out=dd32[:], in0=em[:], in1=r_bc, op=MULT)
                return dd32

            # software pipeline: at iteration ui emit loads for ui+2 and the
            # s2/softmax chain for ui+1, so the PE's wsel block for unit ui
            # overlaps the softmax of ui+1 on DVE/ACT.
            x4s_t = [None] * NU   # x4 tiles in flight
            dd_t = [None] * NU    # dd32 tiles in flight
            x4s_t[0] = emit_load(units[0])
            dd_t[0] = emit_s2(x4s_t[0])
            if NU > 1:
                x4s_t[1] = emit_load(units[1])

            for ui, u in enumerate(units):
                b, q = u
                f0 = q * QF
                if ui + 2 < NU:
                    x4s_t[ui + 2] = emit_load(units[ui + 2])
                if ui + 1 < NU:
                    dd_t[ui + 1] = emit_s2(x4s_t[ui + 1])
                x4 = x4s_t[ui]
                dd32 = dd_t[ui]
                # ---- 4. pdd = rep4.T @ dd32; x4s = x4 * pdd
                pdd = psd_pool.tile([128, TW], f32, tag="pdd")
                nc.tensor.matmul(
                    pdd[:, :], rep4_sb[:], dd32[:], start=True, stop=True
                )
                x4s = x4s_pool.tile([128, TW], bf16, tag="x4s")
                nc.vector.tensor_tensor(
                    out=x4s[:], in0=x4[:], in1=pdd[:], op=MULT
                )
                # ---- 5./6. 16 matmuls in 2-bank pairs + evictions + store
                osb = osb_pool.tile([128, NT, TW], bf16)
                for tg in range(NG):
                    ph = ps_pool.tile([128, 1024], f32, tag="ph")
                    nc.tensor.matmul(
                        ph[:, 0:TW],
                        wsel_sb[:, 2 * tg, :],
                        x4s[:, :],
                        start=True,
                        stop=True,
                    )
                    nc.tensor.matmul(
                        ph[:, 512 : 512 + TW],
                        wsel_sb[:, 2 * tg + 1, :],
                        x4s[:, :],
                        start=True,
                        stop=True,
                    )
                    phv = ph[:, :]
                    src = bass.AP(
                        tensor=phv.tensor,
                        offset=phv.offset,
                        ap=[phv.ap[0], [512, 2], [1, TW]],
                    )
                    dst = osb[:, 2 * tg : 2 * tg + 2, :]
                    if tg in DVE_TGS:
                        nc.vector.tensor_copy(dst, src)
                    else:
                        nc.scalar.copy(dst, src)
                osl = out_d[b, :, f0 : f0 + 1, :]
                dst = bass.AP(
                    tensor=osl.tensor,
                    offset=osl.offset,
                    ap=[[FN, H], [NT * TW, 2], [1, NT * TW]],
                )
                eng = nc.sync if ui % 2 == 0 else nc.scalar
                eng.dma_start(out=dst, in_=osb[:, :, :])
    nc.compile()
    return nc


def _get_nc():
    if "nc" not in _NC_CACHE:
        _NC_CACHE["nc"] = _build_nc()
    return _NC_CACHE["nc"]


def _make_in_maps(x, mask, W, bW, a1, a2, ab):
    bf = ml_dtypes.bfloat16
    x = np.ascontiguousarray(np.asarray(x, np.float32)).astype(bf)
    mask = np.asarray(mask, np.float32)
    W = np.asarray(W, np.float32)
    bW = np.asarray(bW, np.float32)
    a2 = np.asarray(a2, np.float32)

    v = (W @ a2).astype(np.float32)                    # [C]
    md = np.diag(mask).astype(np.float32)              # [N]

    # wsel[row = 32 c + fsub, tp, col = 2 k + jj]:
    #   delta[fsub == tp + 16 jj] * (W[c, k] if c < 3 else bW[k])
    # (column order (k, jj)-interleaved so the store DMA is affine)
    wsel = np.zeros((128, NT, 128), np.float32)
    cols = np.arange(H)
    for tp in range(NT):
        for jj in range(2):
            fsub = tp + 16 * jj
            for c in range(3):
                wsel[32 * c + fsub, tp, 2 * cols + jj] = W[c]
            wsel[96 + fsub, tp, 2 * cols + jj] = bW
    rep4 = np.zeros((NS, 128), np.float32)
    for blk in range(4):
        rep4[:, 32 * blk : 32 * (blk + 1)] = np.eye(NS, dtype=np.float32)
    vsel = np.zeros((128, NS), np.float32)
    for c in range(3):
        vsel[32 * c : 32 * (c + 1), :] = np.eye(NS, dtype=np.float32) * v[c]
    md400 = np.tile(np.tile(md, FSUB)[None, :], (NS, 1)).astype(np.float32)

    wsel = wsel.astype(bf)
    rep4 = rep4.astype(bf)
    vsel = vsel.astype(bf)
    md400 = md400.astype(bf)

    in_maps = []
    for cix in range(NCORES):
        in_maps.append(
            {
                "x": np.ascontiguousarray(x[cix * BPC : (cix + 1) * BPC]),
                "wsel": wsel,
                "rep4": rep4,
                "vsel": vsel,
                "md400": md400,
            }
        )
    return in_maps


def run(x, mask, W, bW, a1, a2, ab, **run_kwargs):
    from concourse.bass_utils import run_bass_kernel_spmd

    nc = _get_nc()
    in_maps = _make_in_maps(x, mask, W, bW, a1, a2, ab)
    res = run_bass_kernel_spmd(nc, in_maps, core_ids=list(range(NCORES)), **run_kwargs)
    out = np.concatenate(
        [np.asarray(res.results[i]["out"]).astype(np.float32) for i in range(NCORES)],
        axis=0,
    )
    return out, res


def kernel(x, mask, W, bW, a1, a2, ab):
    out, _ = run(x, mask, W, bW, a1, a2, ab)
    return out


# revision 10
# speedup vs baseline: 1.5887x; 1.0737x over previous
"""Trainium2 Bass kernel for a GAT block.

Math (after algebraic simplification of the reference):
  h[b,f,n,k] = x[b,:,f,n] @ W[:,k] + bW[k]
  s2[b,f,n]  = h[b,f,n,:] @ a2 = v.x + const   (s1/ab/const cancel in softmax)
  d[b,f,n]   = softmax_n(s2)[n] * mask[n,n]
  out[b,k,f,n] = d[b,f,n] * h[b,f,n,k] = sum_c W[c,k] (x*d)[c,f,n] + bW[k] d[f,n]

Sharding: data-parallel over batch, 4 batches per core on 8 cores.

Device pipeline per (batch, 512-frame q-unit), shapes are [partitions, free]:
  1. x4 [128, 400] bf16: rows 32c+s = x[c] for fsub s (16 frames x 25),
     rows 96:128 = 1.0 (gpsimd memset).
  2. s2p [32, 400] = vsel.T @ x4 on PE (vsel[32c+s, s] = v[c], rows 96+: 0).
  3. softmax: e = exp(s2p) (ACT, PSUM src) -> z = rowsum25 (DVE) ->
     r = 1/z (DVE) -> em = e*md400 (DVE 2x bf16) -> dd32 = em*r_bc (DVE).
  4. pdd [128, 400] = rep4.T @ dd32 (PE replicates d into 4 row blocks);
     x4s = x4 * pdd (DVE, bf16 out): x*d rows 0:96, d rows 96:128.
  5. 16 matmuls, constant-per-tp stationary wsel[tp] [128,128] bf16 (FWL):
     psum rows (2k+jj), cols (fi, n); pairs share a 2-bank psum tile
     [128, 1024] (cols 0:400 and 512:912).
  6. evictions (DVE/ACT mix): strided [128, 2x400] psum -> osb bf16;
     one store per q-unit: osb [128, 6400] -> out[b, :, f0:f0+512, :].

All PE operands are bf16 (fast weight loads); output is bf16, upcast to
fp32 on host (rel err ~4e-3 « 2e-2 tolerance).
"""

import sys

if "/opt/trn_rl_repo" not in sys.path:
    sys.path.insert(0, "/opt/trn_rl_repo")

import numpy as np
import ml_dtypes

B, C, F, N, H = 32, 3, 2048, 25, 64
NCORES = 8
BPC = B // NCORES   # batches per core
QF = 512            # frames per q-unit
NQ = F // QF        # q-units per batch
FSUB = 16           # frames per fsub row
NS = QF // FSUB     # 32 fsub rows per q-unit
FN = F * N
TW = FSUB * N       # 400, columns per tile
NT = NS // 2        # 16 matmuls (of 32 frames) per q-unit
NG = NT // 2        # 8 psum tile-pairs per q-unit

# evictions (of the 8 tile-pairs per unit) routed to DVE; rest go to ACT
DVE_TGS = (1, 4, 6)

_NC_CACHE = {}


def _build_nc():
    import concourse.bass as bass
    import concourse.bacc as bacc
    import concourse.tile as tile
    from concourse import mybir

    f32 = mybir.dt.float32
    bf16 = mybir.dt.bfloat16
    MULT = mybir.AluOpType.mult
    AX = mybir.AxisListType.X
    EXP = mybir.ActivationFunctionType.Exp

    nc = bacc.Bacc()
    x_d = nc.declare_dram_parameter("x", [BPC, C, F, N], bf16, isOutput=False)
    wsel_d = nc.declare_dram_parameter("wsel", [128, NT, 128], bf16, isOutput=False)
    rep4_d = nc.declare_dram_parameter("rep4", [NS, 128], bf16, isOutput=False)
    vsel_d = nc.declare_dram_parameter("vsel", [128, NS], bf16, isOutput=False)
    md_d = nc.declare_dram_parameter("md400", [NS, TW], bf16, isOutput=False)
    out_d = nc.declare_dram_parameter("out", [BPC, H, F, N], bf16, isOutput=True)

    with tile.TileContext(nc) as tc:
        with (
            tc.tile_pool(name="singles", bufs=1) as singles,
            tc.tile_pool(name="x4", bufs=3) as x4_pool,
            tc.tile_pool(name="sm", bufs=3) as sm_pool,
            tc.tile_pool(name="x4s", bufs=2) as x4s_pool,
            tc.tile_pool(name="osb", bufs=3) as osb_pool,
            tc.tile_pool(name="ps", bufs=2, space="PSUM") as ps_pool,
            tc.tile_pool(name="psd", bufs=1, space="PSUM") as psd_pool,
            tc.tile_pool(name="pss", bufs=2, space="PSUM") as pss_pool,
        ):
            wsel_sb = singles.tile([128, NT, 128], bf16)
            nc.sync.dma_start(out=wsel_sb[:], in_=wsel_d[:, :, :])
            rep4_sb = singles.tile([NS, 128], bf16)
            nc.sync.dma_start(out=rep4_sb[:], in_=rep4_d[:, :])
            vsel_sb = singles.tile([128, NS], bf16)
            nc.sync.dma_start(out=vsel_sb[:], in_=vsel_d[:, :])
            md_sb = singles.tile([NS, TW], bf16)
            nc.sync.dma_start(out=md_sb[:], in_=md_d[:, :])

            units = [(b, q) for b in range(BPC) for q in range(NQ)]
            NU = len(units)

            nload = [0]

            def emit_load(u):
                """x4 [128, 400] bf16: rows 0:96 from HBM, rows 96:128 = 1."""
                b, q = u
                f0 = q * QF
                base = x_d[b, :, f0 : f0 + 1, :]  # for offset only
                x4 = x4_pool.tile([128, TW], bf16, tag="x4")
                # rows 96:128 are only ever written here; with a 3-deep pool
                # it suffices to initialize each buffer once
                if nload[0] < 3:
                    nc.vector.memset(x4[96:128, :], 1.0)
                nload[0] += 1
                src4 = bass.AP(
                    tensor=base.tensor,
                    offset=base.offset,
                    ap=[[FN, C], [TW, NS], [1, TW]],
                )
                nc.sync.dma_start(out=x4[0:96, :], in_=src4)
                return x4

            def emit_s2(x4):
                """s2 matmul + softmax chain -> dd32 [32, 400] bf16."""
                s2p = pss_pool.tile([NS, TW], f32, tag="s2p")
                nc.tensor.matmul(
                    s2p[:, :], vsel_sb[:], x4[:], start=True, stop=True
                )
                e = sm_pool.tile([NS, TW], bf16, tag="e")
                nc.scalar.activation(out=e[:], in_=s2p[:], func=EXP)
                ev = e[:].rearrange("p (a b) -> p a b", b=N)
                z = sm_pool.tile([NS, FSUB], f32, tag="z")
                nc.vector.reduce_sum(out=z[:], in_=ev, axis=AX)
                r = sm_pool.tile([NS, FSUB], f32, tag="r")
                nc.vector.reciprocal(out=r[:], in_=z[:])
                em = sm_pool.tile([NS, TW], bf16, tag="em")
                nc.vector.tensor_tensor(out=em[:], in0=e[:], in1=md_sb[:], op=MULT)
                dd32 = sm_pool.tile([NS, TW], bf16, tag="dd32")
                rr = r[:, :]
                r_bc = bass.AP(
                    tensor=rr.tensor,
                    offset=rr.offset,
                    ap=[rr.ap[0], [1, FSUB], [0, N]],
                )
                nc.vector.tensor_tensor(out=dd32[:], in0=em[:], in1=r_bc, op=MULT)
                return dd32

            # software pipeline. Emission order per iteration is chosen so
            # that on each engine's in-order queue the critical ops lead:
            #   PE : rep4(ui), vsel(ui+1), wsel(ui) x16
            #   DVE: x4s(ui) FIRST, then softmax chain(ui+1), then evict(ui)
            #   ACT: exp(ui+1) FIRST, then evict(ui)
            # so the wsel block of unit ui never waits behind chain ops of
            # ui+1 (which themselves wait on the PE).
            x4_t = [None] * NU    # x4 tiles in flight
            dd_t = [None] * NU    # dd32 tiles in flight
            x4_t[0] = emit_load(units[0])
            if NU > 1:
                x4_t[1] = emit_load(units[1])
            dd_t[0] = emit_s2(x4_t[0])

            for ui, u in enumerate(units):
                b, q = u
                f0 = q * QF
                x4 = x4_t[ui]
                dd32 = dd_t[ui]
                # ---- 4. pdd = rep4.T @ dd32; x4s = x4 * pdd
                pdd = psd_pool.tile([128, TW], f32, tag="pdd")
                nc.tensor.matmul(
                    pdd[:, :], rep4_sb[:], dd32[:], start=True, stop=True
                )
                x4s = x4s_pool.tile([128, TW], bf16, tag="x4s")
                nc.vector.tensor_tensor(
                    out=x4s[:], in0=x4[:], in1=pdd[:], op=MULT
                )
                # s2 chain for the NEXT unit: vsel matmul goes to the PE
                # right behind rep4(ui); exp leads the ACT queue; the DVE
                # chain ops sit behind x4s(ui).
                if ui + 1 < NU:
                    dd_t[ui + 1] = emit_s2(x4_t[ui + 1])
                if ui + 2 < NU:
                    x4_t[ui + 2] = emit_load(units[ui + 2])
                # ---- 5./6. 16 matmuls in 2-bank pairs + evictions + store
                osb = osb_pool.tile([128, NT, TW], bf16)
                for tg in range(NG):
                    ph = ps_pool.tile([128, 1024], f32, tag="ph")
                    nc.tensor.matmul(
                        ph[:, 0:TW],
                        wsel_sb[:, 2 * tg, :],
                        x4s[:, :],
                        start=True,
                        stop=True,
                    )
                    nc.tensor.matmul(
                        ph[:, 512 : 512 + TW],
                        wsel_sb[:, 2 * tg + 1, :],
                        x4s[:, :],
                        start=True,
                        stop=True,
                    )
                    phv = ph[:, :]
                    src = bass.AP(
                        tensor=phv.tensor,
                        offset=phv.offset,
                        ap=[phv.ap[0], [512, 2], [1, TW]],
                    )
                    dst = osb[:, 2 * tg : 2 * tg + 2, :]
                    if tg in DVE_TGS:
                        nc.vector.tensor_copy(dst, src)
                    else:
                        nc.scalar.copy(dst, src)
                osl = out_d[b, :, f0 : f0 + 1, :]
                dst = bass.AP(
                    tensor=osl.tensor,
                    offset=osl.offset,
                    ap=[[FN, H], [NT * TW, 2], [1, NT * TW]],
                )
                eng = nc.sync if ui % 2 == 0 else nc.scalar
                eng.dma_start(out=dst, in_=osb[:, :, :])
    nc.compile()
    return nc


def _get_nc():
    if "nc" not in _NC_CACHE:
        _NC_CACHE["nc"] = _build_nc()
    return _NC_CACHE["nc"]


def _make_in_maps(x, mask, W, bW, a1, a2, ab):
    bf = ml_dtypes.bfloat16
    x = np.ascontiguousarray(np.asarray(x, np.float32)).astype(bf)
    mask = np.asarray(mask, np.float32)
    W = np.asarray(W, np.float32)
    bW = np.asarray(bW, np.float32)
    a2 = np.asarray(a2, np.float32)

    v = (W @ a2).astype(np.float32)                    # [C]
    md = np.diag(mask).astype(np.float32)              # [N]

    # wsel[row = 32 c + fsub, tp, col = 2 k + jj]:
    #   delta[fsub == tp + 16 jj] * (W[c, k] if c < 3 else bW[k])
    # (column order (k, jj)-interleaved so the store DMA is affine)
    wsel = np.zeros((128, NT, 128), np.float32)
    cols = np.arange(H)
    for tp in range(NT):
        for jj in range(2):
            fsub = tp + 16 * jj
            for c in range(3):
                wsel[32 * c + fsub, tp, 2 * cols + jj] = W[c]
            wsel[96 + fsub, tp, 2 * cols + jj] = bW
    rep4 = np.zeros((NS, 128), np.float32)
    for blk in range(4):
        rep4[:, 32 * blk : 32 * (blk + 1)] = np.eye(NS, dtype=np.float32)
    vsel = np.zeros((128, NS), np.float32)
    for c in range(3):
        vsel[32 * c : 32 * (c + 1), :] = np.eye(NS, dtype=np.float32) * v[c]
    md400 = np.tile(np.tile(md, FSUB)[None, :], (NS, 1)).astype(np.float32)

    wsel = wsel.astype(bf)
    rep4 = rep4.astype(bf)
    vsel = vsel.astype(bf)
    md400 = md400.astype(bf)

    in_maps = []
    for cix in range(NCORES):
        in_maps.append(
            {
                "x": np.ascontiguousarray(x[cix * BPC : (cix + 1) * BPC]),
                "wsel": wsel,
                "rep4": rep4,
                "vsel": vsel,
                "md400": md400,
            }
        )
    return in_maps


def run(x, mask, W, bW, a1, a2, ab, **run_kwargs):
    from concourse.bass_utils import run_bass_kernel_spmd

    nc = _get_nc()
    in_maps = _make_in_maps(x, mask, W, bW, a1, a2, ab)
    res = run_bass_kernel_spmd(nc, in_maps, core_ids=list(range(NCORES)), **run_kwargs)
    out = np.concatenate(
        [np.asarray(res.results[i]["out"]).astype(np.float32) for i in range(NCORES)],
        axis=0,
    )
    return out, res


def kernel(x, mask, W, bW, a1, a2, ab):
    out, _ = run(x, mask, W, bW, a1, a2, ab)
    return out


# revision 13
# speedup vs baseline: 1.9952x; 1.2559x over previous
"""Trainium2 Bass kernel for a GAT block.

Math (after algebraic simplification of the reference):
  h[b,f,n,k] = x[b,:,f,n] @ W[:,k] + bW[k]
  s2[b,f,n]  = h[b,f,n,:] @ a2 = v.x + const   (s1/ab/const cancel in softmax)
  d[b,f,n]   = softmax_n(s2)[n] * mask[n,n]
  out[b,k,f,n] = d[b,f,n] * h[b,f,n,k] = sum_c W[c,k] (x*d)[c,f,n] + bW[k] d[f,n]

Sharding: data-parallel over batch, 4 batches per core on 8 cores.

Layout: one batch (2048 frames) = 4 interleaved q-units. SBUF partition
32c+s holds frames [64s, 64s+64) of channel c (3.2KB DMA descriptors);
q-unit g covers frames {64s+16g .. 64s+16g+16} = column slice
[400g, 400g+400) of the batch tile. fsub s of unit g = 16 frames.

Device pipeline per q-unit, shapes are [partitions, free]:
  1. x16 [128, 1600] bf16 per batch: rows 32c+s = x[c], rows 96:128 = 1.0.
  2. s2p [32, 400] = vsel.T @ x4 on PE (vsel[32c+s, s] = v[c], rows 96+: 0).
  3. softmax: e = exp(s2p) (ACT) -> z = rowsum25 (DVE) -> r = 1/z (DVE)
     -> em = e*md400 (DVE 2x bf16) -> dd32 = em*r_bc (DVE).
  4. pdd [128, 400] = rep4.T @ dd32 (PE); x4s = x4 * pdd (DVE, bf16):
     x*d rows 0:96, d rows 96:128.
  5. 16 matmuls, stationary wsel[tp] [128,128] bf16 (FWL): psum rows
     (2k+jj); pairs share a 2-bank psum tile [128, 1024].
  6. evictions (DVE/ACT mix): strided [128, 2x400] psum -> osb slot
     4*tp+g; one store per batch: osb [128, 25600] (25.6KB descriptors).

Output bf16, upcast to fp32 on host (rel err ~8e-3 « 2e-2 tolerance).
"""

import sys

if "/opt/trn_rl_repo" not in sys.path:
    sys.path.insert(0, "/opt/trn_rl_repo")

import numpy as np
import ml_dtypes

B, C, F, N, H = 32, 3, 2048, 25, 64
NCORES = 8
BPC = B // NCORES   # batches per core
G = 4               # interleaved q-units per batch
QF = F // G         # 512 frames per q-unit
FSUB = 16           # frames per fsub row (per unit)
NS = QF // FSUB     # 32 fsub rows
FN = F * N
TW = FSUB * N       # 400, columns per unit tile
BW = G * TW         # 1600, columns per batch tile
NT = NS // 2        # 16 matmuls (of 32 frames) per q-unit
NG = NT // 2        # 8 psum tile-pairs per q-unit

# evictions (of the 8 tile-pairs per unit) routed to DVE; rest go to ACT
DVE_TGS = (1, 4, 6)

_NC_CACHE = {}


def _build_nc():
    import concourse.bass as bass
    import concourse.bacc as bacc
    import concourse.tile as tile
    from concourse import mybir

    f32 = mybir.dt.float32
    bf16 = mybir.dt.bfloat16
    MULT = mybir.AluOpType.mult
    AX = mybir.AxisListType.X
    EXP = mybir.ActivationFunctionType.Exp

    nc = bacc.Bacc()
    x_d = nc.declare_dram_parameter("x", [BPC, C, F, N], bf16, isOutput=False)
    wsel_d = nc.declare_dram_parameter("wsel", [128, NT, 128], bf16, isOutput=False)
    rep4_d = nc.declare_dram_parameter("rep4", [NS, 128], bf16, isOutput=False)
    vsel_d = nc.declare_dram_parameter("vsel", [128, NS], bf16, isOutput=False)
    md_d = nc.declare_dram_parameter("md400", [NS, TW], bf16, isOutput=False)
    out_d = nc.declare_dram_parameter("out", [BPC, H, F, N], bf16, isOutput=True)

    with tile.TileContext(nc) as tc:
        with (
            tc.tile_pool(name="singles", bufs=1) as singles,
            tc.tile_pool(name="x16", bufs=2) as x16_pool,
            tc.tile_pool(name="sm", bufs=3) as sm_pool,
            tc.tile_pool(name="x4s", bufs=2) as x4s_pool,
            tc.tile_pool(name="osb", bufs=2) as osb_pool,
            tc.tile_pool(name="ps", bufs=3, space="PSUM") as ps_pool,
            tc.tile_pool(name="psd", bufs=1, space="PSUM") as psd_pool,
            tc.tile_pool(name="pss", bufs=1, space="PSUM") as pss_pool,
        ):
            wsel_sb = singles.tile([128, NT, 128], bf16)
            nc.sync.dma_start(out=wsel_sb[:], in_=wsel_d[:, :, :])
            rep4_sb = singles.tile([NS, 128], bf16)
            nc.sync.dma_start(out=rep4_sb[:], in_=rep4_d[:, :])
            vsel_sb = singles.tile([128, NS], bf16)
            nc.sync.dma_start(out=vsel_sb[:], in_=vsel_d[:, :])
            md_sb = singles.tile([NS, TW], bf16)
            nc.sync.dma_start(out=md_sb[:], in_=md_d[:, :])

            NU = BPC * G        # 16 q-units per core
            nload = [0]

            def emit_load(b):
                """x16 [128, 1600] bf16 for batch b: rows 0:96 from HBM."""
                base = x_d[b, :, 0:1, :]  # for offset only
                x16 = x16_pool.tile([128, BW], bf16, tag="x16")
                # rows 96:128 are only ever written here; with a 2-deep pool
                # it suffices to initialize each buffer once
                if nload[0] < 2:
                    nc.vector.memset(x16[96:128, :], 1.0)
                nload[0] += 1
                src = bass.AP(
                    tensor=base.tensor,
                    offset=base.offset,
                    ap=[[FN, C], [BW, NS], [1, BW]],
                )
                nc.sync.dma_start(out=x16[0:96, :], in_=src)
                return x16

            def x4_view(x16, g, p0=0, p1=128):
                """[p0:p1, 400] unit-g column slice of the batch tile."""
                v = x16[p0:p1, g * TW : (g + 1) * TW]
                return v

            def emit_s2(x16, g):
                """s2 matmul + softmax chain -> dd32 [32, 400] bf16."""
                s2p = pss_pool.tile([NS, TW], f32, tag="s2p")
                nc.tensor.matmul(
                    s2p[:, :], vsel_sb[:], x4_view(x16, g), start=True, stop=True
                )
                e = sm_pool.tile([NS, TW], bf16, tag="e")
                nc.scalar.activation(out=e[:], in_=s2p[:], func=EXP)
                ev = e[:].rearrange("p (a b) -> p a b", b=N)
                z = sm_pool.tile([NS, FSUB], f32, tag="z")
                nc.vector.reduce_sum(out=z[:], in_=ev, axis=AX)
                r = sm_pool.tile([NS, FSUB], f32, tag="r")
                nc.vector.reciprocal(out=r[:], in_=z[:])
                em = sm_pool.tile([NS, TW], bf16, tag="em")
                nc.vector.tensor_tensor(out=em[:], in0=e[:], in1=md_sb[:], op=MULT)
                dd32 = sm_pool.tile([NS, TW], bf16, tag="dd32")
                rr = r[:, :]
                r_bc = bass.AP(
                    tensor=rr.tensor,
                    offset=rr.offset,
                    ap=[rr.ap[0], [1, FSUB], [0, N]],
                )
                nc.vector.tensor_tensor(out=dd32[:], in0=em[:], in1=r_bc, op=MULT)
                return dd32

            # software pipeline. Emission order per iteration keeps the
            # critical ops at the head of each engine's in-order queue:
            #   PE : rep4(ui), vsel(ui+1), wsel(ui) x16
            #   DVE: x4s(ui) first, then chain(ui+1), then evict(ui)
            #   ACT: exp(ui+1) first, then evict(ui)
            x16_t = [None] * (BPC + 1)
            dd_t = [None] * NU
            x16_t[0] = emit_load(0)
            dd_t[0] = emit_s2(x16_t[0], 0)
            osb = None

            for ui in range(NU):
                b, g = divmod(ui, G)
                x16 = x16_t[b]
                dd32 = dd_t[ui]
                if g == 0:
                    osb = osb_pool.tile([128, G * NT, TW], bf16)
                # ---- 4. pdd = rep4.T @ dd32; x4s = x4 * pdd
                pdd = psd_pool.tile([128, TW], f32, tag="pdd")
                nc.tensor.matmul(
                    pdd[:, :], rep4_sb[:], dd32[:], start=True, stop=True
                )
                x4s = x4s_pool.tile([128, TW], bf16, tag="x4s")
                nc.vector.tensor_tensor(
                    out=x4s[:], in0=x4_view(x16, g), in1=pdd[:], op=MULT
                )
                # s2 chain for the NEXT unit
                if ui + 1 < NU:
                    bn, gn = divmod(ui + 1, G)
                    dd_t[ui + 1] = emit_s2(x16_t[bn], gn)
                if g == 0 and b + 1 <= BPC - 1:
                    x16_t[b + 1] = emit_load(b + 1)
                # ---- 5./6. 16 matmuls in 2-bank pairs + evictions
                osv = osb[:, :, :]
                for tg in range(NG):
                    ph = ps_pool.tile([128, 1024], f32, tag="ph")
                    nc.tensor.matmul(
                        ph[:, 0:TW],
                        wsel_sb[:, 2 * tg, :],
                        x4s[:, :],
                        start=True,
                        stop=True,
                    )
                    nc.tensor.matmul(
                        ph[:, 512 : 512 + TW],
                        wsel_sb[:, 2 * tg + 1, :],
                        x4s[:, :],
                        start=True,
                        stop=True,
                    )
                    phv = ph[:, :]
                    src = bass.AP(
                        tensor=phv.tensor,
                        offset=phv.offset,
                        ap=[phv.ap[0], [512, 2], [1, TW]],
                    )
                    # output slots for (unit g, pair tp=2tg,2tg+1):
                    # 4*tp+g and 4*(tp+1)+g -> stride 4*TW
                    dst = bass.AP(
                        tensor=osv.tensor,
                        offset=osv.offset + (8 * tg + g) * TW,
                        ap=[osv.ap[0], [G * TW, 2], [1, TW]],
                    )
                    if tg in DVE_TGS:
                        nc.vector.tensor_copy(dst, src)
                    else:
                        nc.scalar.copy(dst, src)
                # ---- one store per batch
                if g == G - 1:
                    osl = out_d[b, :, 0:1, :]
                    dst = bass.AP(
                        tensor=osl.tensor,
                        offset=osl.offset,
                        ap=[[FN, H], [G * NT * TW, 2], [1, G * NT * TW]],
                    )
                    eng = nc.sync if b % 2 == 0 else nc.scalar
                    eng.dma_start(out=dst, in_=osb[:, :, :])
    nc.compile()
    return nc


def _get_nc():
    if "nc" not in _NC_CACHE:
        _NC_CACHE["nc"] = _build_nc()
    return _NC_CACHE["nc"]


def _make_in_maps(x, mask, W, bW, a1, a2, ab):
    bf = ml_dtypes.bfloat16
    x = np.ascontiguousarray(np.asarray(x, np.float32)).astype(bf)
    mask = np.asarray(mask, np.float32)
    W = np.asarray(W, np.float32)
    bW = np.asarray(bW, np.float32)
    a2 = np.asarray(a2, np.float32)

    v = (W @ a2).astype(np.float32)                    # [C]
    md = np.diag(mask).astype(np.float32)              # [N]

    # wsel[row = 32 c + fsub, tp, col = 2 k + jj]:
    #   delta[fsub == tp + 16 jj] * (W[c, k] if c < 3 else bW[k])
    # (column order (k, jj)-interleaved so the store DMA is affine)
    wsel = np.zeros((128, NT, 128), np.float32)
    cols = np.arange(H)
    for tp in range(NT):
        for jj in range(2):
            fsub = tp + 16 * jj
            for c in range(3):
                wsel[32 * c + fsub, tp, 2 * cols + jj] = W[c]
            wsel[96 + fsub, tp, 2 * cols + jj] = bW
    rep4 = np.zeros((NS, 128), np.float32)
    for blk in range(4):
        rep4[:, 32 * blk : 32 * (blk + 1)] = np.eye(NS, dtype=np.float32)
    vsel = np.zeros((128, NS), np.float32)
    for c in range(3):
        vsel[32 * c : 32 * (c + 1), :] = np.eye(NS, dtype=np.float32) * v[c]
    md400 = np.tile(np.tile(md, FSUB)[None, :], (NS, 1)).astype(np.float32)

    wsel = wsel.astype(bf)
    rep4 = rep4.astype(bf)
    vsel = vsel.astype(bf)
    md400 = md400.astype(bf)

    in_maps = []
    for cix in range(NCORES):
        in_maps.append(
            {
                "x": np.ascontiguousarray(x[cix * BPC : (cix + 1) * BPC]),
                "wsel": wsel,
                "rep4": rep4,
                "vsel": vsel,
                "md400": md400,
            }
        )
    return in_maps


def run(x, mask, W, bW, a1, a2, ab, **run_kwargs):
    from concourse.bass_utils import run_bass_kernel_spmd

    nc = _get_nc()
    in_maps = _make_in_maps(x, mask, W, bW, a1, a2, ab)
    res = run_bass_kernel_spmd(nc, in_maps, core_ids=list(range(NCORES)), **run_kwargs)
    out = np.concatenate(
        [np.asarray(res.results[i]["out"]).astype(np.float32) for i in range(NCORES)],
        axis=0,
    )
    return out, res


def kernel(x, mask, W, bW, a1, a2, ab):
    out, _ = run(x, mask, W, bW, a1, a2, ab)
    return out
